# revision 62
# baseline (speedup 1.0000x reference)
"""Self-contained Trainium2 Bass kernel for the sparse point-attention module.

Strategy: shard the point dimension n across the 8 NeuronCores (512 points
each, both batch entries on every core).  Each core gets the full `pos`
(tiny) so the KNN is purely local; everything else is data-parallel and no
collectives are needed.

v7 -- 432us HW (vs 676-892us for the v2 baseline).  The three changes
that mattered, each found from perfetto/NTFF trace analysis:

1. Depth-2 software pipeline.  v2 ran each chunk's stages serially, so
   the PE waited 2-4us on the DVE's a1in product every chunk and the HAM
   clock-gate re-throttled the PE to 1.2GHz ~40% of the run.  Emission
   is now:  iteration gc = [front-end(gc+1): pe1/kq/v/pe2 matmuls +
   evictions + DVE a1in] [a1(gc) + a1r evictions 6ACT/2DVE] [a2(gc-1) +
   exp + softmax tail] [knn piece pacing] [kv prefetch gc+3].  a1 inputs
   are computed one iteration ahead and a2 trails one iteration behind
   its a1, so neither the DVE product nor the a1r evictions are ever on
   the PE's critical path (gaps >400ns: 225us -> ~35us, throttle 40%->5%).

2. ap_gather restructure.  A channels=16 ap_gather costs ~28ns PER INDEX
   on ONE gpsimd core (~15us per 512-idx chunk, invisible in per-op
   durations!) and separate gathers do NOT overlap across cores.  v2's
   per-chunk gathers were a hidden ~460us serial floor.  Now ONE
   channels=QT gather per KNN tile (num_idxs=256) runs all 8 cores
   concurrently (~7.5us/tile): per-core index blocks idxM[16j+p, m] =
   idxt16[p, 16j+m] are built by NBLK tiny same-shape SBUF DMAs, and the
   blocked [QT, 256] output is re-stitched into a flat [16, QT*16] bf16
   strip by NBLK casting DMAs (gpsimd-triggered) that pe1/kq slice at
   base partition 0 (matmul rhs base must be 0/32/64; ap_gather idx APs
   at partition offsets silently mis-read on HW -- both found the hard
   way).

3. KNN idx transpose on the PE.  v2's HWDGE transposing DMA took ~14us
   per tile and serialized every tile boundary.  The top-16 indices are
   extracted as fp32 VALUES (exact to 2^24) and transposed by a single
   identity matmul in transpose mode into PSUM partitions 0-15, then
   evicted as i16 (~0.5us total).

Negative results worth remembering: fp8e4 DoubleRow for a1 is
speed-NEUTRAL (the moving operand still streams 1 element/cycle; it only
halves the pass count, so 8x1024-el MMs == 16x512-el MMs) while costing
~1.5e-2 branch rel err; moving the v+pe add from the PE-accumulate trick
to DVE or GPSIMD regresses (the elementwise engines/SBUF ports are the
contended resource, PE slots are not); per-chunk ACT/DVE op times inflate
15-20% under full load (SBUF port contention), so keep SBUF traffic off
the busy engines.

KNN core is unchanged from v2: centered-distance hi/lo bf16 matmul with
fp16 eviction into the HIGH halves of a persistent u32 array whose LOW
halves hold a one-time iota16; max8 over the fp32 view gives values AND
indices; two-level 512-group scan.  q-conv elimination, bias folds (bp2
rides a ones-row of pe1c, ba2 dropped, bv folded into the output bias)
also unchanged.
"""

import numpy as np
import ml_dtypes

BF16 = ml_dtypes.bfloat16
# ml_dtypes.float8_e4m3 is the IEEE-style e4m3 (max normal 240, has
# infinities) -- exactly TRN's FP8_EXP4, and what mybir.dt.np maps to
FP8 = ml_dtypes.float8_e4m3

# ---- problem dimensions (hardcoded, must match the grader's inputs) ----
B = 2
CIN = 128
N = 4096
KK = 16          # neighbours
DIM = 256
PHID = 64
AHID = 1024
NCORES = 8
NLOC = N // NCORES
BN_EPS = 1e-5
NEG_BIG = -1e30


def _dims_full():
    return dict(B=B, CIN=CIN, N=N, KK=KK, DIM=DIM, PHID=PHID, AHID=AHID,
                NLOC=NLOC)


def build_nc(dims):
    """Build the (single, SPMD) Bass program for one core's shard."""
    import concourse.bass as bass
    import concourse.mybir as mybir
    import concourse.tile as tile
    from concourse import bacc
    from concourse.bass import ts

    fp32 = mybir.dt.float32
    bf16 = mybir.dt.bfloat16
    fp16 = mybir.dt.float16
    u16 = mybir.dt.uint16
    i16 = mybir.dt.int16
    u32 = mybir.dt.uint32
    AF = mybir.ActivationFunctionType
    OP = mybir.AluOpType
    AX = mybir.AxisListType

    Bn = dims["B"]; CINn = dims["CIN"]; Nn = dims["N"]; KKn = dims["KK"]
    DIMn = dims["DIM"]; PHIDn = dims["PHID"]; AHIDn = dims["AHID"]
    NLOCn = dims["NLOC"]

    QT = min(128, NLOCn)              # queries per KNN tile
    NQT = NLOCn // QT                 # KNN tiles per batch
    CHUNK = 512                       # matmul column chunk (n,k cols)
    CQ = CHUNK // KKn                 # queries per chunk (32)
    NCH_TILE = (QT * KKn) // CHUNK    # chunks per KNN tile
    NCH_D = Nn // 512                 # 512-col chunks of the distance row
    NCHB = NQT * NCH_TILE             # chunks per batch
    DM = DIMn // 128                  # feature tiles (2)
    AM = AHIDn // 128                 # a-hidden tiles (8)
    KA1 = DIMn // 128                 # contraction tiles for a1 (2)
    GRP = 512                         # KNN L1 group size
    NGRP = Nn // GRP                  # 8 for the full problem
    TWO_LEVEL = NGRP >= 4             # direct scan for the small sim config
    PF = 3                            # k/v DMA prefetch depth (chunks)

    nc = bacc.Bacc()

    # ---- DRAM parameters ----
    key_r = nc.declare_dram_parameter("key_r", [Bn, CINn, NLOCn * KKn], bf16, isOutput=False)
    val_r = nc.declare_dram_parameter("val_r", [Bn, CINn, NLOCn * KKn], bf16, isOutput=False)
    paug_lhs = nc.declare_dram_parameter("paug_lhs", [Bn, 13, NLOCn], bf16, isOutput=False)
    paug_rhs = nc.declare_dram_parameter("paug_rhs", [Bn, 13, Nn], bf16, isOutput=False)
    pos16_d = nc.declare_dram_parameter("pos16", [Bn, QT, Nn], fp32, isOutput=False)
    iota_d = nc.declare_dram_parameter("iota32", [QT, Nn], u32, isOutput=False)
    WkTn_d = nc.declare_dram_parameter("WkTn", [CINn, DIMn], bf16, isOutput=False)
    WvT_d = nc.declare_dram_parameter("WvT", [CINn, DIMn], bf16, isOutput=False)
    WqTb_d = nc.declare_dram_parameter("WqTb", [4, DIMn], bf16, isOutput=False)
    Wp1q_d = nc.declare_dram_parameter("Wp1q", [4, PHIDn], bf16, isOutput=False)
    Wp2T_d = nc.declare_dram_parameter("Wp2T", [PHIDn + 1, DIMn], bf16, isOutput=False)
    Wa1T_d = nc.declare_dram_parameter("Wa1T", [128, KA1, AHIDn], bf16, isOutput=False)
    Wa2T_d = nc.declare_dram_parameter("Wa2T", [128, AM, DIMn], bf16, isOutput=False)
    WeT_d = nc.declare_dram_parameter("WeT", [128, DM, DIMn], bf16, isOutput=False)
    ba1_d = nc.declare_dram_parameter("ba1f", [128, AM], fp32, isOutput=False)
    be_d = nc.declare_dram_parameter("bef", [128, DM], fp32, isOutput=False)
    out_d = nc.declare_dram_parameter("out", [Bn, DIMn, NLOCn], fp32, isOutput=True)

    with tile.TileContext(nc) as tc:
        with (
            tc.tile_pool(name="wpool", bufs=1) as wpool,
            tc.tile_pool(name="bpool", bufs=2) as bpool,
            tc.tile_pool(name="kpool", bufs=2) as kpool,
            tc.tile_pool(name="kvpool", bufs=PF + 1) as kvpool,
            tc.tile_pool(name="gpool", bufs=3) as gpool,
            tc.tile_pool(name="cpool", bufs=2) as cpool,
            tc.tile_pool(name="ypool", bufs=2) as ypool,
            tc.tile_pool(name="pspool", bufs=4, space="PSUM") as pspool,
            tc.tile_pool(name="kqpool", bufs=1, space="PSUM") as kqpool,
            tc.tile_pool(name="a2pool", bufs=1, space="PSUM") as a2pool,
        ):
            # ---- load weights / constants once.  The KNN-critical
            # tensors (paug, iota, pos16) go FIRST on the DMA queue: the
            # prologue's tile-0 KNN->gather chain is the serial startup
            # path and must not sit behind ~1.5MB of MLP weights. ----
            from concourse import masks
            ident = wpool.tile([128, 128], fp32, tag="ident")
            masks.make_identity(nc, ident[:])

            WkTn = wpool.tile([CINn, DIMn], bf16)
            WvT = wpool.tile([CINn, DIMn], bf16)
            WqTb = wpool.tile([4, DIMn], bf16)
            Wp1q = wpool.tile([4, PHIDn], bf16)
            Wp2T = wpool.tile([PHIDn + 1, DIMn], bf16)
            Wa1T = wpool.tile([128, KA1, AHIDn], bf16)
            Wa2T = wpool.tile([128, AM, DIMn], bf16)
            WeT = wpool.tile([128, DM, DIMn], bf16)
            ba1f = wpool.tile([128, AM], fp32)
            bef = wpool.tile([128, DM], fp32)

            # packed distance array: hi u16 = fp16 dneg, lo u16 = iota
            dsb32 = wpool.tile([QT, Nn], u32, tag="dsb32")
            dsb_f32 = dsb32[:].bitcast(fp32)
            dsb_hi = dsb32[:].bitcast(fp16).rearrange(
                "p (n two) -> p n two", two=2)

            def iota_fill():
                # one contiguous u32 load: lo halves = iota, hi halves = 0
                # (the hi halves are overwritten by the distance evictions
                # before any scan reads them)
                nc.sync.dma_start(out=dsb32[:], in_=iota_d[:])

            # pe1 activations with a trailing ones-row (bias row of Wp2T);
            # two buffers so front-end(c+1) never waits on pe2(c) reads
            pe1cs = []
            for _ in range(2):
                p1c = bpool.tile([PHIDn + 1, CHUNK], bf16, tag="pe1c")
                nc.vector.memset(p1c[PHIDn:PHIDn + 1, :], 1.0)
                pe1cs.append(p1c)

            prhs_sbs, plhs_sbs, pos16s = [], [], []
            for b in range(Bn):
                prhs_sb = bpool.tile([13, Nn], bf16, tag="prhs_sb")
                nc.sync.dma_start(out=prhs_sb[:], in_=paug_rhs[b])
                plhs_sb = bpool.tile([13, NLOCn], bf16, tag="plhs_sb")
                nc.sync.dma_start(out=plhs_sb[:], in_=paug_lhs[b])
                pos16 = bpool.tile([QT, Nn], fp32, tag="pos16")
                pos16s.append(pos16)
                prhs_sbs.append(prhs_sb); plhs_sbs.append(plhs_sb)

            iota_fill()
            # DMA order tracks the prologue critical path: batch-0 pos
            # table (tile-0 gather), then the weights (front_end(0)),
            # then the batch-1 table (not needed for ~150us)
            nc.sync.dma_start(out=pos16s[0][:], in_=pos16_d[0])
            for sb, dr in [(WkTn, WkTn_d), (WvT, WvT_d), (WqTb, WqTb_d),
                           (Wp1q, Wp1q_d), (Wp2T, Wp2T_d), (Wa1T, Wa1T_d),
                           (Wa2T, Wa2T_d), (WeT, WeT_d),
                           (ba1f, ba1_d), (bef, be_d)]:
                nc.sync.dma_start(out=sb[:], in_=dr[:])
            if Bn > 1:
                nc.sync.dma_start(out=pos16s[1][:], in_=pos16_d[1])

            NCAND = NGRP * 8 if TWO_LEVEL else Nn

            def knn_start(b, t):
                """Emit-piece list for one KNN tile (distances + top-16).

                The per-(query,k) indices are extracted as fp32 VALUES in
                idxf32 [QT queries, 16] and transposed on the PE (identity
                matmul in transpose mode, exact for integers <= 4095) into
                PSUM [16, QT] at partitions 0-15, evicted as i16, then
                scattered into per-core blocks idxM[16j+p, m] =
                idxt16[p, 16j+m] with NBLK tiny same-shape SBUF DMAs.  ONE
                ap_gather (channels=QT, num_idxs=256) then gathers the
                whole tile's neighbour positions with all QT/16 gpsimd
                cores working concurrently (ap_gather wall time is
                num_idxs x ~28ns PER CORE-GROUP and separate gathers do
                NOT overlap -- per-chunk channels=16 gathers cost ~15us
                each and were the real serial bottleneck).  The blocked
                [QT, 256] result is re-stitched to a flat [16, QT*16]
                bf16 strip (cast riding the DMA) that the pe1/q matmuls
                slice at base partition 0.
                """
                cand = kpool.tile([QT, max(NCAND, 16) if TWO_LEVEL else 8],
                                  fp32, tag="cand")
                v8a = kpool.tile([QT, 8], fp32, tag="v8a")
                v8b = kpool.tile([QT, 8], fp32, tag="v8b")
                idxf32 = kpool.tile([QT, 16], fp32, tag="idxf32")
                idxt16 = kpool.tile([16, QT], i16, tag="idxt16")
                idxM = kpool.tile([QT, 16], i16, tag="idxM")
                posgT = kpool.tile([QT, 16 * KKn], fp32, tag="posgT")
                posgbS = kpool.tile([16, QT * KKn], bf16, tag="posgbS")

                def p_d(lo, hi):
                    for nch in range(lo, hi):
                        dps = pspool.tile([128, 512], fp32, tag="ps")
                        nc.tensor.matmul(
                            dps[0:QT, :], plhs_sbs[b][:, ts(t, QT)],
                            prhs_sbs[b][:, ts(nch, 512)])
                        # fp16 eviction into the u32 HIGH halves, split
                        # ACT/DVE so the 8-eviction burst clears the pspool
                        # banks fast enough that the next tile's front-end
                        # never bank-stalls at the boundary
                        dst = dsb_hi[:, nch * 512:(nch + 1) * 512, 1:2]
                        if nch % 2 == 0:
                            nc.scalar.activation(dst, dps[0:QT, :], AF.Copy)
                        else:
                            nc.vector.tensor_copy(dst, dps[0:QT, :])

                def ext(v8, lohi):
                    # low u16 of each packed fp32 = original column index,
                    # converted to fp32 VALUES for the PE transpose
                    nc.vector.tensor_copy(
                        idxf32[:, lohi * 8:(lohi + 1) * 8],
                        v8[:].bitcast(u16).rearrange(
                            "p (k two) -> p k two", two=2)[:, :, 0:1])

                def fin():
                    tps = pspool.tile([128, 512], fp32, tag="ps")
                    nc.tensor.transpose(tps[0:16, 0:QT], idxf32[:],
                                        ident[0:QT, 0:QT])
                    nc.vector.tensor_copy(idxt16[:], tps[0:16, 0:QT])
                    NBLK = QT // 16
                    BCOL = 16 * KKn
                    # idx-scatter DMAs trigger from gpsimd: they precede
                    # the gather in its queue anyway, and the sync queue's
                    # trigger+semaphore-reuse serialization delayed the
                    # prologue's tile-0 chain by ~6us
                    for j in range(NBLK):
                        nc.gpsimd.dma_start(out=idxM[16 * j:16 * j + 16, :],
                                            in_=idxt16[0:16, 16 * j:16 * j + 16])
                    nc.gpsimd.ap_gather(
                        posgT[:], pos16s[b][:], idxM[:],
                        channels=QT, num_elems=Nn, d=1, num_idxs=BCOL)
                    for j in range(NBLK):
                        nc.gpsimd.dma_start(
                            out=posgbS[0:16, BCOL * j:BCOL * (j + 1)],
                            in_=posgT[16 * j:16 * j + 16, :])

                if TWO_LEVEL:
                    def p_l1(lo, hi):
                        for g in range(lo, hi):
                            nc.vector.max(
                                out=cand[:, g * 8:(g + 1) * 8],
                                in_=dsb_f32[:, g * GRP:(g + 1) * GRP])

                    def p_l2():
                        nc.vector.max(out=v8a[:], in_=cand[:, 0:NCAND])
                        ext(v8a, 0)
                        nc.vector.match_replace(
                            out=cand[:, 0:NCAND], in_to_replace=v8a[:],
                            in_values=cand[:, 0:NCAND], imm_value=NEG_BIG)
                        nc.vector.max(out=v8b[:], in_=cand[:, 0:NCAND])
                        ext(v8b, 1)
                        fin()

                    pieces = [lambda: p_d(0, NCH_D),
                              lambda: p_l1(0, NGRP),
                              p_l2]
                else:
                    def p_small():
                        nc.vector.max(out=v8a[:], in_=dsb_f32[:])
                        ext(v8a, 0)
                        nc.vector.match_replace(
                            out=dsb_f32[:], in_to_replace=v8a[:],
                            in_values=dsb_f32[:], imm_value=NEG_BIG)
                        nc.vector.max(out=v8b[:], in_=dsb_f32[:])
                        ext(v8b, 1)
                        fin()
                        # restore the iota lows that match_replace clobbered
                        iota_fill()

                    pieces = [lambda: p_d(0, NCH_D), p_small]

                return posgbS, pieces

            # ---- k/v chunk prefetch (rolling, PF chunks ahead) ----
            NGC = Bn * NCHB                    # global chunk count
            kv_bufs = {}

            def kv_prefetch(g):
                if g >= NGC:
                    return
                bb = g // NCHB
                cc = g % NCHB
                col0 = cc * CHUNK
                kbf = kvpool.tile([CINn, CHUNK], bf16, tag="kbf")
                vbf = kvpool.tile([CINn, CHUNK], bf16, tag="vbf")
                # sync queue: its semaphore-reuse waits must not head-block
                # the gathers (gpsimd) or anything else latency-critical
                nc.sync.dma_start(out=kbf[:],
                                  in_=key_r[bb, :, col0:col0 + CHUNK])
                nc.sync.dma_start(out=vbf[:],
                                  in_=val_r[bb, :, col0:col0 + CHUNK])
                kv_bufs[g] = (kbf, vbf)

            posgb_list = [None] * (Bn * NQT)

            # ---- per-chunk pipeline stages ----
            state = {}
            aggsbs = {}

            def front_end(gc):
                """pe1/pe2/kq/v+pe matmuls + evictions + DVE a1in for gc."""
                cc_t = gc % NCH_TILE
                col0 = cc_t * CHUNK
                posgb = posgb_list[gc // NCH_TILE][0:4, col0:col0 + CHUNK]
                kbf, vbf = kv_bufs.pop(gc)
                pe1c = pe1cs[gc % 2]
                # pe1 = relu((Wp1f Wq) pos + b) straight from pos
                p1ps = pspool.tile([128, 512], fp32, tag="ps")
                nc.tensor.matmul(p1ps[0:PHIDn, :], Wp1q[:], posgb)
                nc.scalar.activation(pe1c[0:PHIDn, :], p1ps[0:PHIDn, :],
                                     AF.Relu)
                # rr = q - k_f + 1 accumulated on the PE; emitted between
                # the pe1 matmul and the pe1c consumers so the PE never
                # waits on the relu eviction
                kqps = kqpool.tile([128, DM, CHUNK], fp32, tag="kq")
                for m in range(DM):
                    nc.tensor.matmul(kqps[:, m, :], WqTb[:, ts(m, 128)],
                                     posgb, start=True, stop=False)
                    nc.tensor.matmul(kqps[:, m, :], WkTn[:, ts(m, 128)],
                                     kbf[:], start=False, stop=True)
                vpss = []
                for m in range(DM):
                    vps = pspool.tile([128, 512], fp32, tag="ps")
                    nc.tensor.matmul(vps[:], WvT[:, ts(m, 128)],
                                     vbf[:], start=True, stop=False)
                    vpss.append(vps)
                # pe = Wp2 pe1 + bp2 (bias row rides the matmul); evicted
                # bf16 (walrus only allows ONE PSUM operand per DVE op, so
                # the a1in product needs one SBUF side)
                peg = cpool.tile([128, DM, CHUNK], bf16, tag="peg")
                for m in range(DM):
                    p2ps = pspool.tile([128, 512], fp32, tag="ps")
                    nc.tensor.matmul(p2ps[:], Wp2T[:, ts(m, 128)], pe1c[:])
                    nc.scalar.activation(peg[:, m, :], p2ps[:], AF.Copy)
                # vpe = v + pe accumulated on the PE (Wp2 re-run into the
                # v PSUM; bp2 rides the ones-row): cheaper in practice than
                # a DVE/gpsimd add (measured -- the elementwise engines are
                # the contended resource, PE slots are not).  3-deep ring:
                # last read (evpe) is two iterations later.
                vpe = gpool.tile([128, DM, CHUNK], bf16, tag="vpe")
                for m in range(DM):
                    nc.tensor.matmul(vpss[m][:], Wp2T[:, ts(m, 128)],
                                     pe1c[:], start=False, stop=True)
                    nc.scalar.activation(vpe[:, m, :], vpss[m][:], AF.Copy)
                # a1in on the DVE, consumed one iteration later
                a1in = cpool.tile([128, KA1, CHUNK], bf16, tag="a1in")
                nc.vector.tensor_mul(a1in[:], kqps[:], peg[:])
                state[gc] = (a1in, vpe)

            def back_a1(gc):
                """a1 matmuls + relu evictions for chunk gc.  The a2 stage
                runs one iteration LATER (back_a2) so the a1r evictions
                always have a full iteration of slack -- the PE never
                stalls on the eviction engines mid-chunk."""
                a1in, vpe = state.pop(gc)
                a1r = cpool.tile([128, AM, CHUNK], bf16, tag="a1r")
                for mt in range(AM):
                    a1ps = pspool.tile([128, CHUNK], fp32, tag="ps")
                    for kt in range(KA1):
                        nc.tensor.matmul(
                            a1ps[:], Wa1T[:, kt, ts(mt, 128)],
                            a1in[:, kt, :],
                            start=(kt == 0), stop=(kt == KA1 - 1))
                    if mt % 4 == 3:
                        nc.vector.tensor_scalar(
                            a1r[:, mt, :], a1ps[:],
                            ba1f[:, mt:mt + 1], 0.0,
                            op0=OP.add, op1=OP.max)
                    else:
                        nc.scalar.activation(a1r[:, mt, :], a1ps[:],
                                             AF.Relu,
                                             bias=ba1f[:, mt:mt + 1])
                state[("a1r", gc)] = (a1r, vpe)

            def back_a2(gc):
                """a2 matmuls + exp + softmax tail for chunk gc."""
                b = gc // NCHB
                cc = gc % NCHB
                a1r, vpe = state.pop(("a1r", gc))
                if cc == 0:
                    aggsb = bpool.tile([128, DM, NLOCn], bf16, tag="aggsb")
                    aggsbs[b] = aggsb
                aggsb = aggsbs[b]
                a2ps = a2pool.tile([128, DM, CHUNK], fp32, tag="a2")
                for m in range(DM):
                    for kt in range(AM):
                        nc.tensor.matmul(
                            a2ps[:, m, :], Wa2T[:, kt, ts(m, 128)],
                            a1r[:, kt, :],
                            start=(kt == 0), stop=(kt == AM - 1))
                ee = cpool.tile([128, DM, CHUNK], bf16, tag="ee")
                nc.scalar.activation(ee[:], a2ps[:], AF.Exp)
                # all-2-byte contiguous APs for DVE 2x modes
                evpe = cpool.tile([128, DM, CHUNK], bf16, tag="evpe")
                nc.vector.tensor_mul(evpe[:], ee[:], vpe[:])
                sums = cpool.tile([128, 2, DM * CQ], bf16, tag="sums")
                with nc.allow_low_precision("16-way softmax sums"):
                    nc.vector.tensor_reduce(
                        sums[:, 0, :],
                        ee[:].rearrange("p m (g k) -> p (m g) k", k=KKn),
                        axis=AX.X, op=OP.add)
                    nc.vector.tensor_reduce(
                        sums[:, 1, :],
                        evpe[:].rearrange("p m (g k) -> p (m g) k",
                                          k=KKn),
                        axis=AX.X, op=OP.add)
                erec = cpool.tile([128, DM * CQ], fp32, tag="erec")
                nc.vector.reciprocal(erec[:], sums[:, 0, :])
                col_lo = cc * CQ
                nc.vector.tensor_mul(
                    aggsb[:, :, col_lo:col_lo + CQ],
                    sums[:, 1, :].rearrange("p (m g) -> p m g", m=DM),
                    erec[:].rearrange("p (m g) -> p m g", m=DM))

            def final_conv(b):
                aggsb = aggsbs.pop(b)
                for nloc0 in range(0, NLOCn, 512):
                    w = min(512, NLOCn - nloc0)
                    for m in range(DM):
                        yps = pspool.tile([128, 512], fp32, tag="ps")
                        for kt in range(DM):
                            nc.tensor.matmul(
                                yps[:, :w], WeT[:, kt, ts(m, 128)],
                                aggsb[:, kt, nloc0:nloc0 + w],
                                start=(kt == 0), stop=(kt == DM - 1))
                        yev = ypool.tile([128, 512], fp32, tag="yev")
                        nc.vector.tensor_scalar_add(yev[:, :w], yps[:, :w],
                                                    bef[:, m:m + 1])
                        nc.sync.dma_start(
                            out=out_d[b, ts(m, 128), nloc0:nloc0 + w],
                            in_=yev[:, :w])

            # ---- prologue ----
            tiles = [(b, t) for b in range(Bn) for t in range(NQT)]
            for g in range(PF):
                kv_prefetch(g)
            posgb_list[0], pieces0 = knn_start(*tiles[0])
            for p in pieces0:
                p()
            if len(tiles) > 1:
                posgb_list[1], pieces = knn_start(*tiles[1])
            else:
                pieces = []
            piece_i = [0]

            def run_piece(n=1):
                stop = min(len(pieces), piece_i[0] + n)
                while piece_i[0] < stop:
                    pieces[piece_i[0]]()
                    piece_i[0] += 1

            run_piece(1)          # p_d of tile 1 up front
            front_end(0)

            # ---- main pipelined loop ----
            # KNN pacing for tile T = ti+1 during tile ti (pieces =
            # [p_d, l1, l2+gather-chain]): the distance MMs were emitted
            # at (T-2, c_last) right after knn_start so the PE crosses the
            # tile boundary with dense work; the 8 L1 scans sit at the c0
            # DVE tail; l2 + PE idx transpose + idx-scatter DMAs + the
            # single per-tile ap_gather + stitch DMAs go at c1 START --
            # the ~13us gather chain then completes with a full iteration
            # of slack before front_end(T, c0) at (ti, c3) needs it.
            for gc in range(NGC):
                ti = gc // NCH_TILE
                c = gc % NCH_TILE
                if c == 1:
                    run_piece(len(pieces))      # l2 + gather chain
                if gc + 1 < NGC:
                    front_end(gc + 1)
                back_a1(gc)
                if gc > 0:
                    back_a2(gc - 1)
                if gc % NCHB == 1 and gc > NCHB:
                    final_conv(gc // NCHB - 1)
                kv_prefetch(gc + PF)
                if c == 0:
                    run_piece(1)                # l1 scans at the DVE tail
                if c == NCH_TILE - 1:
                    run_piece(len(pieces))      # safety flush
                    if ti + 2 < len(tiles):
                        posgb_list[ti + 2], pieces = knn_start(
                            *tiles[ti + 2])
                        piece_i[0] = 0
                        run_piece(1)            # p_d at the PE tail
                    else:
                        pieces = []
                        piece_i[0] = 0

            # epilogue: last chunk's a2+tail + last batch's output conv
            back_a2(NGC - 1)
            final_conv(Bn - 1)

    nc.finalize()   # Bacc.finalize: wait legalization, library loads, ISA codegen
    return nc


def host_prepare(inputs, dims, ncores=NCORES):
    """Fold BN/biases into weights, pre-transpose for the PE, shard by n."""
    d = dims
    f32 = np.float32
    key = np.asarray(inputs["key"], f32)
    values = np.asarray(inputs["values"], f32)
    pos = np.asarray(inputs["pos"], f32)
    g = lambda n: np.asarray(inputs[n], f32)

    Wk, bk = g("Wk"), g("bk")
    Wq, bq = g("Wq"), g("bq")
    Wv, bv = g("Wv"), g("bv")
    Wp1, bp1 = g("Wp1"), g("bp1")
    Wp2, bp2 = g("Wp2"), g("bp2")
    Wa1, ba1 = g("Wa1"), g("ba1")
    Wa2 = g("Wa2")
    We, be = g("We"), g("be")

    p_sc = g("p_gamma") / np.sqrt(g("p_var") + f32(BN_EPS))
    Wp1f = (Wp1 * p_sc[:, None]).astype(f32)
    bp1f = (bp1 * p_sc + g("p_beta") - g("p_mean") * p_sc).astype(f32)
    a_sc = g("a_gamma") / np.sqrt(g("a_var") + f32(BN_EPS))
    Wa1f = (Wa1 * a_sc[:, None]).astype(f32)
    ba1f = (ba1 * a_sc + g("a_beta") - g("a_mean") * a_sc).astype(f32)

    DM = d["DIM"] // 128
    AM = d["AHID"] // 128
    KA1 = d["DIM"] // 128
    QT = min(128, d["NLOC"])

    def colsplit(v, nt):  # (nt*128,) -> (128, nt)
        return np.ascontiguousarray(v.reshape(nt, 128).T).astype(f32)

    # pe1 = relu(Wp1f q + bp1f) with q = Wq pos + bq folds to a 4-row conv
    Wp1q = np.concatenate(
        [(Wp1f @ Wq).T, (Wp1f @ bq + bp1f)[None, :]], 0)          # (4, PHID)
    Wp2Tb = np.concatenate([Wp2.T, bp2[None, :]], 0)              # (PHID+1, DIM)

    common = {
        "WkTn": np.ascontiguousarray(-Wk.T).astype(BF16),
        "WvT": np.ascontiguousarray(Wv.T).astype(BF16),
        "WqTb": np.ascontiguousarray(np.concatenate(
            [Wq.T, (bq - bk + 1.0)[None, :]], 0)).astype(BF16),
        "Wp1q": np.ascontiguousarray(Wp1q).astype(BF16),
        "Wp2T": np.ascontiguousarray(Wp2Tb).astype(BF16),
        "Wa1T": np.ascontiguousarray(
            Wa1f.T.reshape(KA1, 128, d["AHID"]).transpose(1, 0, 2)).astype(BF16),
        "Wa2T": np.ascontiguousarray(
            Wa2.T.reshape(AM, 128, d["DIM"]).transpose(1, 0, 2)).astype(BF16),
        "WeT": np.ascontiguousarray(
            We.T.reshape(DM, 128, d["DIM"]).transpose(1, 0, 2)).astype(BF16),
        "ba1f": colsplit(ba1f, AM),
        "bef": colsplit((We @ bv + be).astype(f32), DM),
        "iota32": np.ascontiguousarray(
            np.broadcast_to(np.arange(d["N"], dtype=np.uint32)[None, :],
                            (QT, d["N"]))),
    }

    # hi/lo bf16 split of pos, |p_j|^2 and |p_i|^2 for the centered distance
    # matmul: dneg ~= -d  (top values near 0 -> fp16 eviction is precise)
    sq = (pos * pos).sum(axis=1).astype(f32)              # (B, N)
    pos_hi = pos.astype(BF16)
    pos_lo = (pos - pos_hi.astype(f32)).astype(BF16)
    sq_hi = sq.astype(BF16)
    sq_lo = (sq - sq_hi.astype(f32)).astype(BF16)
    paug_rhs = np.concatenate(
        [2.0 * pos_hi.astype(f32), 2.0 * pos_lo.astype(f32),
         2.0 * pos_hi.astype(f32), -sq_hi.astype(f32)[:, None, :],
         -sq_lo.astype(f32)[:, None, :],
         -np.ones((d["B"], 2, d["N"]), f32)], 1).astype(BF16)
    # (pos, 1) replicated into every 16-partition block so each gpsimd
    # core's gather block reads its own copy of the table
    pos16 = np.zeros((d["B"], QT, d["N"]), f32)
    for j in range(QT // 16):
        pos16[:, 16 * j:16 * j + 3] = pos
        pos16[:, 16 * j + 3] = 1.0

    in_maps = []
    for cid in range(ncores):
        n0 = cid * d["NLOC"]
        n1 = n0 + d["NLOC"]
        m = dict(common)
        m["key_r"] = np.ascontiguousarray(key[:, :, n0:n1, :]).reshape(
            d["B"], d["CIN"], d["NLOC"] * d["KK"]).astype(BF16)
        m["val_r"] = np.ascontiguousarray(values[:, :, n0:n1, :]).reshape(
            d["B"], d["CIN"], d["NLOC"] * d["KK"]).astype(BF16)
        m["paug_lhs"] = np.ascontiguousarray(np.concatenate(
            [pos_hi.astype(f32)[:, :, n0:n1], pos_hi.astype(f32)[:, :, n0:n1],
             pos_lo.astype(f32)[:, :, n0:n1],
             np.ones((d["B"], 2, d["NLOC"]), f32),
             sq_hi.astype(f32)[:, None, n0:n1],
             sq_lo.astype(f32)[:, None, n0:n1]], 1)).astype(BF16)
        m["paug_rhs"] = paug_rhs
        m["pos16"] = pos16
        in_maps.append(m)
    return in_maps


_NC_CACHE = {}


def _get_nc(dims_key):
    if dims_key not in _NC_CACHE:
        _NC_CACHE[dims_key] = build_nc(_dims_full())
    return _NC_CACHE[dims_key]


def kernel(**inputs):
    from concourse.bass_utils import run_bass_kernel_spmd
    dims = _dims_full()
    nc = _get_nc("full")
    in_maps = host_prepare(inputs, dims)
    res = run_bass_kernel_spmd(nc, in_maps, core_ids=list(range(NCORES)))
    outs = [r["out"].astype(np.float32) for r in res.results]
    return np.concatenate(outs, axis=2)


# revision 64
# speedup vs baseline: 1.0656x; 1.0656x over previous
"""Self-contained Trainium2 Bass kernel for the sparse point-attention module.

Strategy: shard the point dimension n across the 8 NeuronCores (512 points
each, both batch entries on every core).  Each core gets the full `pos`
(tiny) so the KNN is purely local; everything else is data-parallel and no
collectives are needed.

v7 -- 432us HW (vs 676-892us for the v2 baseline).  The three changes
that mattered, each found from perfetto/NTFF trace analysis:

1. Depth-2 software pipeline.  v2 ran each chunk's stages serially, so
   the PE waited 2-4us on the DVE's a1in product every chunk and the HAM
   clock-gate re-throttled the PE to 1.2GHz ~40% of the run.  Emission
   is now:  iteration gc = [front-end(gc+1): pe1/kq/v/pe2 matmuls +
   evictions + DVE a1in] [a1(gc) + a1r evictions 6ACT/2DVE] [a2(gc-1) +
   exp + softmax tail] [knn piece pacing] [kv prefetch gc+3].  a1 inputs
   are computed one iteration ahead and a2 trails one iteration behind
   its a1, so neither the DVE product nor the a1r evictions are ever on
   the PE's critical path (gaps >400ns: 225us -> ~35us, throttle 40%->5%).

2. ap_gather restructure.  A channels=16 ap_gather costs ~28ns PER INDEX
   on ONE gpsimd core (~15us per 512-idx chunk, invisible in per-op
   durations!) and separate gathers do NOT overlap across cores.  v2's
   per-chunk gathers were a hidden ~460us serial floor.  Now ONE
   channels=QT gather per KNN tile (num_idxs=256) runs all 8 cores
   concurrently (~7.5us/tile): per-core index blocks idxM[16j+p, m] =
   idxt16[p, 16j+m] are built by NBLK tiny same-shape SBUF DMAs, and the
   blocked [QT, 256] output is re-stitched into a flat [16, QT*16] bf16
   strip by NBLK casting DMAs (gpsimd-triggered) that pe1/kq slice at
   base partition 0 (matmul rhs base must be 0/32/64; ap_gather idx APs
   at partition offsets silently mis-read on HW -- both found the hard
   way).

3. KNN idx transpose on the PE.  v2's HWDGE transposing DMA took ~14us
   per tile and serialized every tile boundary.  The top-16 indices are
   extracted as fp32 VALUES (exact to 2^24) and transposed by a single
   identity matmul in transpose mode into PSUM partitions 0-15, then
   evicted as i16 (~0.5us total).

Negative results worth remembering: fp8e4 DoubleRow for a1 is
speed-NEUTRAL (the moving operand still streams 1 element/cycle; it only
halves the pass count, so 8x1024-el MMs == 16x512-el MMs) while costing
~1.5e-2 branch rel err; moving the v+pe add from the PE-accumulate trick
to DVE or GPSIMD regresses (the elementwise engines/SBUF ports are the
contended resource, PE slots are not); per-chunk ACT/DVE op times inflate
15-20% under full load (SBUF port contention), so keep SBUF traffic off
the busy engines.

KNN core is unchanged from v2: centered-distance hi/lo bf16 matmul with
fp16 eviction into the HIGH halves of a persistent u32 array whose LOW
halves hold a one-time iota16; max8 over the fp32 view gives values AND
indices; two-level 512-group scan.  q-conv elimination, bias folds (bp2
rides a ones-row of pe1c, ba2 dropped, bv folded into the output bias)
also unchanged.
"""

import numpy as np
import ml_dtypes

BF16 = ml_dtypes.bfloat16
# ml_dtypes.float8_e4m3 is the IEEE-style e4m3 (max normal 240, has
# infinities) -- exactly TRN's FP8_EXP4, and what mybir.dt.np maps to
FP8 = ml_dtypes.float8_e4m3

# ---- problem dimensions (hardcoded, must match the grader's inputs) ----
B = 2
CIN = 128
N = 4096
KK = 16          # neighbours
DIM = 256
PHID = 64
AHID = 1024
NCORES = 8
NLOC = N // NCORES
BN_EPS = 1e-5
NEG_BIG = -1e30


def _dims_full():
    return dict(B=B, CIN=CIN, N=N, KK=KK, DIM=DIM, PHID=PHID, AHID=AHID,
                NLOC=NLOC)


def build_nc(dims):
    """Build the (single, SPMD) Bass program for one core's shard."""
    import concourse.bass as bass
    import concourse.mybir as mybir
    import concourse.tile as tile
    from concourse import bacc
    from concourse.bass import ts

    fp32 = mybir.dt.float32
    bf16 = mybir.dt.bfloat16
    fp16 = mybir.dt.float16
    u16 = mybir.dt.uint16
    i16 = mybir.dt.int16
    u32 = mybir.dt.uint32
    AF = mybir.ActivationFunctionType
    OP = mybir.AluOpType
    AX = mybir.AxisListType

    Bn = dims["B"]; CINn = dims["CIN"]; Nn = dims["N"]; KKn = dims["KK"]
    DIMn = dims["DIM"]; PHIDn = dims["PHID"]; AHIDn = dims["AHID"]
    NLOCn = dims["NLOC"]

    QT = min(128, NLOCn)              # queries per KNN tile
    NQT = NLOCn // QT                 # KNN tiles per batch
    CHUNK = 512                       # matmul column chunk (n,k cols)
    CQ = CHUNK // KKn                 # queries per chunk (32)
    NCH_TILE = (QT * KKn) // CHUNK    # chunks per KNN tile
    NCH_D = Nn // 512                 # 512-col chunks of the distance row
    NCHB = NQT * NCH_TILE             # chunks per batch
    DM = DIMn // 128                  # feature tiles (2)
    AM = AHIDn // 128                 # a-hidden tiles (8)
    KA1 = DIMn // 128                 # contraction tiles for a1 (2)
    GRP = 512                         # KNN L1 group size
    NGRP = Nn // GRP                  # 8 for the full problem
    TWO_LEVEL = NGRP >= 4             # direct scan for the small sim config
    PF = 3                            # k/v DMA prefetch depth (chunks)

    nc = bacc.Bacc()

    # ---- DRAM parameters ----
    key_r = nc.declare_dram_parameter("key_r", [Bn, CINn, NLOCn * KKn], bf16, isOutput=False)
    val_r = nc.declare_dram_parameter("val_r", [Bn, CINn, NLOCn * KKn], bf16, isOutput=False)
    paug_lhs = nc.declare_dram_parameter("paug_lhs", [Bn, 13, NLOCn], bf16, isOutput=False)
    paug_rhs = nc.declare_dram_parameter("paug_rhs", [Bn, 13, Nn], bf16, isOutput=False)
    pos16_d = nc.declare_dram_parameter("pos16", [Bn, QT, Nn], fp32, isOutput=False)
    iota_d = nc.declare_dram_parameter("iota32", [QT, Nn], u32, isOutput=False)
    WkTn_d = nc.declare_dram_parameter("WkTn", [CINn, DIMn], bf16, isOutput=False)
    WvT_d = nc.declare_dram_parameter("WvT", [CINn, DIMn], bf16, isOutput=False)
    WqTb_d = nc.declare_dram_parameter("WqTb", [4, DIMn], bf16, isOutput=False)
    Wp1q_d = nc.declare_dram_parameter("Wp1q", [4, PHIDn], bf16, isOutput=False)
    Wp2T_d = nc.declare_dram_parameter("Wp2T", [PHIDn + 1, DIMn], bf16, isOutput=False)
    Wa1T_d = nc.declare_dram_parameter("Wa1T", [128, KA1, AHIDn], bf16, isOutput=False)
    Wa2T_d = nc.declare_dram_parameter("Wa2T", [128, AM, DIMn], bf16, isOutput=False)
    WeT_d = nc.declare_dram_parameter("WeT", [128, DM, DIMn], bf16, isOutput=False)
    ba1_d = nc.declare_dram_parameter("ba1f", [128, AM], fp32, isOutput=False)
    be_d = nc.declare_dram_parameter("bef", [128, DM], fp32, isOutput=False)
    out_d = nc.declare_dram_parameter("out", [Bn, DIMn, NLOCn], fp32, isOutput=True)

    with tile.TileContext(nc) as tc:
        with (
            tc.tile_pool(name="wpool", bufs=1) as wpool,
            tc.tile_pool(name="bpool", bufs=2) as bpool,
            tc.tile_pool(name="kpool", bufs=2) as kpool,
            tc.tile_pool(name="kvpool", bufs=PF + 1) as kvpool,
            tc.tile_pool(name="gpool", bufs=3) as gpool,
            tc.tile_pool(name="cpool", bufs=2) as cpool,
            tc.tile_pool(name="ypool", bufs=2) as ypool,
            tc.tile_pool(name="pspool", bufs=4, space="PSUM") as pspool,
            tc.tile_pool(name="kqpool", bufs=1, space="PSUM") as kqpool,
            tc.tile_pool(name="a2pool", bufs=1, space="PSUM") as a2pool,
        ):
            # ---- load weights / constants once.  The KNN-critical
            # tensors (paug, iota, pos16) go FIRST on the DMA queue: the
            # prologue's tile-0 KNN->gather chain is the serial startup
            # path and must not sit behind ~1.5MB of MLP weights. ----
            from concourse import masks
            ident = wpool.tile([128, 128], fp32, tag="ident")
            masks.make_identity(nc, ident[:])

            WkTn = wpool.tile([CINn, DIMn], bf16)
            WvT = wpool.tile([CINn, DIMn], bf16)
            WqTb = wpool.tile([4, DIMn], bf16)
            Wp1q = wpool.tile([4, PHIDn], bf16)
            Wp2T = wpool.tile([PHIDn + 1, DIMn], bf16)
            Wa1T = wpool.tile([128, KA1, AHIDn], bf16)
            Wa2T = wpool.tile([128, AM, DIMn], bf16)
            WeT = wpool.tile([128, DM, DIMn], bf16)
            ba1f = wpool.tile([128, AM], fp32)
            bef = wpool.tile([128, DM], fp32)

            # packed distance array: hi u16 = fp16 dneg, lo u16 = iota
            dsb32 = wpool.tile([QT, Nn], u32, tag="dsb32")
            dsb_f32 = dsb32[:].bitcast(fp32)
            dsb_hi = dsb32[:].bitcast(fp16).rearrange(
                "p (n two) -> p n two", two=2)

            def iota_fill():
                # one contiguous u32 load: lo halves = iota, hi halves = 0
                # (the hi halves are overwritten by the distance evictions
                # before any scan reads them)
                nc.sync.dma_start(out=dsb32[:], in_=iota_d[:])

            # pe1 activations with a trailing ones-row (bias row of Wp2T);
            # two buffers so front-end(c+1) never waits on pe2(c) reads
            pe1cs = []
            for _ in range(2):
                p1c = bpool.tile([PHIDn + 1, CHUNK], bf16, tag="pe1c")
                nc.vector.memset(p1c[PHIDn:PHIDn + 1, :], 1.0)
                pe1cs.append(p1c)

            prhs_sbs, plhs_sbs, pos16s = [], [], []
            for b in range(Bn):
                prhs_sb = bpool.tile([13, Nn], bf16, tag="prhs_sb")
                nc.sync.dma_start(out=prhs_sb[:], in_=paug_rhs[b])
                plhs_sb = bpool.tile([13, NLOCn], bf16, tag="plhs_sb")
                nc.sync.dma_start(out=plhs_sb[:], in_=paug_lhs[b])
                pos16 = bpool.tile([QT, Nn], fp32, tag="pos16")
                pos16s.append(pos16)
                prhs_sbs.append(prhs_sb); plhs_sbs.append(plhs_sb)

            iota_fill()
            # DMA order tracks the prologue critical path: batch-0 pos
            # table (tile-0 gather), then the weights (front_end(0)),
            # then the batch-1 table (not needed for ~150us)
            nc.sync.dma_start(out=pos16s[0][:], in_=pos16_d[0])
            for sb, dr in [(WkTn, WkTn_d), (WvT, WvT_d), (WqTb, WqTb_d),
                           (Wp1q, Wp1q_d), (Wp2T, Wp2T_d), (Wa1T, Wa1T_d),
                           (Wa2T, Wa2T_d), (WeT, WeT_d),
                           (ba1f, ba1_d), (bef, be_d)]:
                nc.sync.dma_start(out=sb[:], in_=dr[:])
            if Bn > 1:
                nc.sync.dma_start(out=pos16s[1][:], in_=pos16_d[1])

            NCAND = NGRP * 8 if TWO_LEVEL else Nn

            def knn_start(b, t):
                """Emit-piece list for one KNN tile (distances + top-16).

                The per-(query,k) indices are extracted as fp32 VALUES in
                idxf32 [QT queries, 16] and transposed on the PE (identity
                matmul in transpose mode, exact for integers <= 4095) into
                PSUM [16, QT] at partitions 0-15, evicted as i16, then
                scattered into per-core blocks idxM[16j+p, m] =
                idxt16[p, 16j+m] with NBLK tiny same-shape SBUF DMAs.  ONE
                ap_gather (channels=QT, num_idxs=256) then gathers the
                whole tile's neighbour positions with all QT/16 gpsimd
                cores working concurrently (ap_gather wall time is
                num_idxs x ~28ns PER CORE-GROUP and separate gathers do
                NOT overlap -- per-chunk channels=16 gathers cost ~15us
                each and were the real serial bottleneck).  The blocked
                [QT, 256] result is re-stitched to a flat [16, QT*16]
                bf16 strip (cast riding the DMA) that the pe1/q matmuls
                slice at base partition 0.
                """
                cand = kpool.tile([QT, max(NCAND, 16) if TWO_LEVEL else 8],
                                  fp32, tag="cand")
                v8a = kpool.tile([QT, 8], fp32, tag="v8a")
                v8b = kpool.tile([QT, 8], fp32, tag="v8b")
                idxf32 = kpool.tile([QT, 16], fp32, tag="idxf32")
                idxt16 = kpool.tile([16, QT], i16, tag="idxt16")
                idxM = kpool.tile([QT, 16], i16, tag="idxM")
                posgT = kpool.tile([QT, 16 * KKn], fp32, tag="posgT")
                posgbS = kpool.tile([16, QT * KKn], bf16, tag="posgbS")

                def p_d(lo, hi):
                    for nch in range(lo, hi):
                        dps = pspool.tile([128, 512], fp32, tag="ps")
                        nc.tensor.matmul(
                            dps[0:QT, :], plhs_sbs[b][:, ts(t, QT)],
                            prhs_sbs[b][:, ts(nch, 512)])
                        # fp16 eviction into the u32 HIGH halves (ACT only:
                        # DVE-side evictions head-block the next iteration's
                        # a1in product -- measured +28us)
                        dst = dsb_hi[:, nch * 512:(nch + 1) * 512, 1:2]
                        nc.scalar.activation(dst, dps[0:QT, :], AF.Copy)

                def ext(v8, lohi):
                    # low u16 of each packed fp32 = original column index,
                    # converted to fp32 VALUES for the PE transpose
                    nc.vector.tensor_copy(
                        idxf32[:, lohi * 8:(lohi + 1) * 8],
                        v8[:].bitcast(u16).rearrange(
                            "p (k two) -> p k two", two=2)[:, :, 0:1])

                def fin():
                    tps = pspool.tile([128, 512], fp32, tag="ps")
                    nc.tensor.transpose(tps[0:16, 0:QT], idxf32[:],
                                        ident[0:QT, 0:QT])
                    nc.vector.tensor_copy(idxt16[:], tps[0:16, 0:QT])
                    NBLK = QT // 16
                    BCOL = 16 * KKn
                    # idx-scatter DMAs stay on the sync queue (scalar and
                    # gpsimd triggering both measured slower: they
                    # head-block evictions / the gather respectively)
                    for j in range(NBLK):
                        nc.sync.dma_start(out=idxM[16 * j:16 * j + 16, :],
                                          in_=idxt16[0:16, 16 * j:16 * j + 16])
                    nc.gpsimd.ap_gather(
                        posgT[:], pos16s[b][:], idxM[:],
                        channels=QT, num_elems=Nn, d=1, num_idxs=BCOL)
                    for j in range(NBLK):
                        nc.gpsimd.dma_start(
                            out=posgbS[0:16, BCOL * j:BCOL * (j + 1)],
                            in_=posgT[16 * j:16 * j + 16, :])

                if TWO_LEVEL:
                    def p_l1(lo, hi):
                        for g in range(lo, hi):
                            nc.vector.max(
                                out=cand[:, g * 8:(g + 1) * 8],
                                in_=dsb_f32[:, g * GRP:(g + 1) * GRP])

                    def p_l2():
                        nc.vector.max(out=v8a[:], in_=cand[:, 0:NCAND])
                        ext(v8a, 0)
                        nc.vector.match_replace(
                            out=cand[:, 0:NCAND], in_to_replace=v8a[:],
                            in_values=cand[:, 0:NCAND], imm_value=NEG_BIG)
                        nc.vector.max(out=v8b[:], in_=cand[:, 0:NCAND])
                        ext(v8b, 1)
                        fin()

                    pieces = [lambda: p_d(0, NCH_D),
                              lambda: p_l1(0, NGRP),
                              p_l2]
                else:
                    def p_small():
                        nc.vector.max(out=v8a[:], in_=dsb_f32[:])
                        ext(v8a, 0)
                        nc.vector.match_replace(
                            out=dsb_f32[:], in_to_replace=v8a[:],
                            in_values=dsb_f32[:], imm_value=NEG_BIG)
                        nc.vector.max(out=v8b[:], in_=dsb_f32[:])
                        ext(v8b, 1)
                        fin()
                        # restore the iota lows that match_replace clobbered
                        iota_fill()

                    pieces = [lambda: p_d(0, NCH_D), p_small]

                return posgbS, pieces

            # ---- k/v chunk prefetch (rolling, PF chunks ahead) ----
            NGC = Bn * NCHB                    # global chunk count
            kv_bufs = {}

            def kv_prefetch(g):
                if g >= NGC:
                    return
                bb = g // NCHB
                cc = g % NCHB
                col0 = cc * CHUNK
                kbf = kvpool.tile([CINn, CHUNK], bf16, tag="kbf")
                vbf = kvpool.tile([CINn, CHUNK], bf16, tag="vbf")
                # sync queue: its semaphore-reuse waits must not head-block
                # the gathers (gpsimd) or anything else latency-critical
                nc.sync.dma_start(out=kbf[:],
                                  in_=key_r[bb, :, col0:col0 + CHUNK])
                nc.sync.dma_start(out=vbf[:],
                                  in_=val_r[bb, :, col0:col0 + CHUNK])
                kv_bufs[g] = (kbf, vbf)

            posgb_list = [None] * (Bn * NQT)

            # ---- per-chunk pipeline stages ----
            state = {}
            aggsbs = {}

            def front_end(gc):
                """pe1/pe2/kq/v+pe matmuls + evictions + DVE a1in for gc."""
                cc_t = gc % NCH_TILE
                col0 = cc_t * CHUNK
                posgb = posgb_list[gc // NCH_TILE][0:4, col0:col0 + CHUNK]
                kbf, vbf = kv_bufs.pop(gc)
                pe1c = pe1cs[gc % 2]
                # pe1 = relu((Wp1f Wq) pos + b) straight from pos
                p1ps = pspool.tile([128, 512], fp32, tag="ps")
                nc.tensor.matmul(p1ps[0:PHIDn, :], Wp1q[:], posgb)
                nc.scalar.activation(pe1c[0:PHIDn, :], p1ps[0:PHIDn, :],
                                     AF.Relu)
                # rr = q - k_f + 1 accumulated on the PE; emitted between
                # the pe1 matmul and the pe1c consumers so the PE never
                # waits on the relu eviction
                kqps = kqpool.tile([128, DM, CHUNK], fp32, tag="kq")
                for m in range(DM):
                    nc.tensor.matmul(kqps[:, m, :], WqTb[:, ts(m, 128)],
                                     posgb, start=True, stop=False)
                    nc.tensor.matmul(kqps[:, m, :], WkTn[:, ts(m, 128)],
                                     kbf[:], start=False, stop=True)
                vpss = []
                for m in range(DM):
                    vps = pspool.tile([128, 512], fp32, tag="ps")
                    nc.tensor.matmul(vps[:], WvT[:, ts(m, 128)],
                                     vbf[:], start=True, stop=False)
                    vpss.append(vps)
                # pe = Wp2 pe1 + bp2 (bias row rides the matmul); evicted
                # bf16 (walrus only allows ONE PSUM operand per DVE op, so
                # the a1in product needs one SBUF side)
                peg = cpool.tile([128, DM, CHUNK], bf16, tag="peg")
                for m in range(DM):
                    p2ps = pspool.tile([128, 512], fp32, tag="ps")
                    nc.tensor.matmul(p2ps[:], Wp2T[:, ts(m, 128)], pe1c[:])
                    nc.scalar.activation(peg[:, m, :], p2ps[:], AF.Copy)
                # vpe = v + pe accumulated on the PE (Wp2 re-run into the
                # v PSUM; bp2 rides the ones-row): cheaper in practice than
                # a DVE/gpsimd add (measured -- the elementwise engines are
                # the contended resource, PE slots are not).  3-deep ring:
                # last read (evpe) is two iterations later.
                vpe = gpool.tile([128, DM, CHUNK], bf16, tag="vpe")
                for m in range(DM):
                    nc.tensor.matmul(vpss[m][:], Wp2T[:, ts(m, 128)],
                                     pe1c[:], start=False, stop=True)
                    nc.scalar.activation(vpe[:, m, :], vpss[m][:], AF.Copy)
                # a1in on the DVE, consumed one iteration later
                a1in = cpool.tile([128, KA1, CHUNK], bf16, tag="a1in")
                nc.vector.tensor_mul(a1in[:], kqps[:], peg[:])
                state[gc] = (a1in, vpe)

            def back_a1(gc):
                """a1 matmuls + relu evictions for chunk gc.  The a2 stage
                runs one iteration LATER (back_a2) so the a1r evictions
                always have a full iteration of slack -- the PE never
                stalls on the eviction engines mid-chunk."""
                a1in, vpe = state.pop(gc)
                a1r = cpool.tile([128, AM, CHUNK], bf16, tag="a1r")
                for mt in range(AM):
                    a1ps = pspool.tile([128, CHUNK], fp32, tag="ps")
                    for kt in range(KA1):
                        nc.tensor.matmul(
                            a1ps[:], Wa1T[:, kt, ts(mt, 128)],
                            a1in[:, kt, :],
                            start=(kt == 0), stop=(kt == KA1 - 1))
                    if mt % 4 == 3:
                        nc.vector.tensor_scalar(
                            a1r[:, mt, :], a1ps[:],
                            ba1f[:, mt:mt + 1], 0.0,
                            op0=OP.add, op1=OP.max)
                    else:
                        nc.scalar.activation(a1r[:, mt, :], a1ps[:],
                                             AF.Relu,
                                             bias=ba1f[:, mt:mt + 1])
                state[("a1r", gc)] = (a1r, vpe)

            def back_a2(gc):
                """a2 matmuls + exp + softmax tail for chunk gc."""
                b = gc // NCHB
                cc = gc % NCHB
                a1r, vpe = state.pop(("a1r", gc))
                if cc == 0:
                    aggsb = bpool.tile([128, DM, NLOCn], bf16, tag="aggsb")
                    aggsbs[b] = aggsb
                aggsb = aggsbs[b]
                a2ps = a2pool.tile([128, DM, CHUNK], fp32, tag="a2")
                for m in range(DM):
                    for kt in range(AM):
                        nc.tensor.matmul(
                            a2ps[:, m, :], Wa2T[:, kt, ts(m, 128)],
                            a1r[:, kt, :],
                            start=(kt == 0), stop=(kt == AM - 1))
                ee = cpool.tile([128, DM, CHUNK], bf16, tag="ee")
                nc.scalar.activation(ee[:], a2ps[:], AF.Exp)
                # all-2-byte contiguous APs for DVE 2x modes
                evpe = cpool.tile([128, DM, CHUNK], bf16, tag="evpe")
                nc.vector.tensor_mul(evpe[:], ee[:], vpe[:])
                sums = cpool.tile([128, 2, DM * CQ], bf16, tag="sums")
                with nc.allow_low_precision("16-way softmax sums"):
                    nc.vector.tensor_reduce(
                        sums[:, 0, :],
                        ee[:].rearrange("p m (g k) -> p (m g) k", k=KKn),
                        axis=AX.X, op=OP.add)
                    nc.vector.tensor_reduce(
                        sums[:, 1, :],
                        evpe[:].rearrange("p m (g k) -> p (m g) k",
                                          k=KKn),
                        axis=AX.X, op=OP.add)
                erec = cpool.tile([128, DM * CQ], fp32, tag="erec")
                nc.vector.reciprocal(erec[:], sums[:, 0, :])
                col_lo = cc * CQ
                nc.vector.tensor_mul(
                    aggsb[:, :, col_lo:col_lo + CQ],
                    sums[:, 1, :].rearrange("p (m g) -> p m g", m=DM),
                    erec[:].rearrange("p (m g) -> p m g", m=DM))

            def final_conv(b):
                aggsb = aggsbs.pop(b)
                for nloc0 in range(0, NLOCn, 512):
                    w = min(512, NLOCn - nloc0)
                    for m in range(DM):
                        yps = pspool.tile([128, 512], fp32, tag="ps")
                        for kt in range(DM):
                            nc.tensor.matmul(
                                yps[:, :w], WeT[:, kt, ts(m, 128)],
                                aggsb[:, kt, nloc0:nloc0 + w],
                                start=(kt == 0), stop=(kt == DM - 1))
                        yev = ypool.tile([128, 512], fp32, tag="yev")
                        nc.vector.tensor_scalar_add(yev[:, :w], yps[:, :w],
                                                    bef[:, m:m + 1])
                        nc.sync.dma_start(
                            out=out_d[b, ts(m, 128), nloc0:nloc0 + w],
                            in_=yev[:, :w])

            # ---- prologue ----
            tiles = [(b, t) for b in range(Bn) for t in range(NQT)]
            for g in range(PF):
                kv_prefetch(g)
            posgb_list[0], pieces0 = knn_start(*tiles[0])
            for p in pieces0:
                p()
            if len(tiles) > 1:
                posgb_list[1], pieces = knn_start(*tiles[1])
            else:
                pieces = []
            piece_i = [0]

            def run_piece(n=1):
                stop = min(len(pieces), piece_i[0] + n)
                while piece_i[0] < stop:
                    pieces[piece_i[0]]()
                    piece_i[0] += 1

            run_piece(1)          # p_d of tile 1 up front
            front_end(0)

            # ---- main pipelined loop ----
            # KNN pacing for tile T = ti+1 during tile ti (pieces =
            # [p_d, l1, l2+gather-chain]): the distance MMs were emitted
            # at (T-2, c_last) right after knn_start so the PE crosses the
            # tile boundary with dense work; the 8 L1 scans sit at the c0
            # DVE tail; l2 + PE idx transpose + idx-scatter DMAs + the
            # single per-tile ap_gather + stitch DMAs go at c1 START --
            # the ~13us gather chain then completes with a full iteration
            # of slack before front_end(T, c0) at (ti, c3) needs it.
            for gc in range(NGC):
                ti = gc // NCH_TILE
                c = gc % NCH_TILE
                if c == 1:
                    run_piece(len(pieces))      # l2 + gather chain
                if gc + 1 < NGC:
                    front_end(gc + 1)
                back_a1(gc)
                if gc > 0:
                    back_a2(gc - 1)
                if gc % NCHB == 1 and gc > NCHB:
                    final_conv(gc // NCHB - 1)
                kv_prefetch(gc + PF)
                if c == 0:
                    run_piece(1)                # l1 scans at the DVE tail
                if c == NCH_TILE - 1:
                    run_piece(len(pieces))      # safety flush
                    if ti + 2 < len(tiles):
                        posgb_list[ti + 2], pieces = knn_start(
                            *tiles[ti + 2])
                        piece_i[0] = 0
                        run_piece(1)            # p_d at the PE tail
                    else:
                        pieces = []
                        piece_i[0] = 0

            # epilogue: last chunk's a2+tail + last batch's output conv
            back_a2(NGC - 1)
            final_conv(Bn - 1)

    nc.finalize()   # Bacc.finalize: wait legalization, library loads, ISA codegen
    return nc


def host_prepare(inputs, dims, ncores=NCORES):
    """Fold BN/biases into weights, pre-transpose for the PE, shard by n."""
    d = dims
    f32 = np.float32
    key = np.asarray(inputs["key"], f32)
    values = np.asarray(inputs["values"], f32)
    pos = np.asarray(inputs["pos"], f32)
    g = lambda n: np.asarray(inputs[n], f32)

    Wk, bk = g("Wk"), g("bk")
    Wq, bq = g("Wq"), g("bq")
    Wv, bv = g("Wv"), g("bv")
    Wp1, bp1 = g("Wp1"), g("bp1")
    Wp2, bp2 = g("Wp2"), g("bp2")
    Wa1, ba1 = g("Wa1"), g("ba1")
    Wa2 = g("Wa2")
    We, be = g("We"), g("be")

    p_sc = g("p_gamma") / np.sqrt(g("p_var") + f32(BN_EPS))
    Wp1f = (Wp1 * p_sc[:, None]).astype(f32)
    bp1f = (bp1 * p_sc + g("p_beta") - g("p_mean") * p_sc).astype(f32)
    a_sc = g("a_gamma") / np.sqrt(g("a_var") + f32(BN_EPS))
    Wa1f = (Wa1 * a_sc[:, None]).astype(f32)
    ba1f = (ba1 * a_sc + g("a_beta") - g("a_mean") * a_sc).astype(f32)

    DM = d["DIM"] // 128
    AM = d["AHID"] // 128
    KA1 = d["DIM"] // 128
    QT = min(128, d["NLOC"])

    def colsplit(v, nt):  # (nt*128,) -> (128, nt)
        return np.ascontiguousarray(v.reshape(nt, 128).T).astype(f32)

    # pe1 = relu(Wp1f q + bp1f) with q = Wq pos + bq folds to a 4-row conv
    Wp1q = np.concatenate(
        [(Wp1f @ Wq).T, (Wp1f @ bq + bp1f)[None, :]], 0)          # (4, PHID)
    Wp2Tb = np.concatenate([Wp2.T, bp2[None, :]], 0)              # (PHID+1, DIM)

    common = {
        "WkTn": np.ascontiguousarray(-Wk.T).astype(BF16),
        "WvT": np.ascontiguousarray(Wv.T).astype(BF16),
        "WqTb": np.ascontiguousarray(np.concatenate(
            [Wq.T, (bq - bk + 1.0)[None, :]], 0)).astype(BF16),
        "Wp1q": np.ascontiguousarray(Wp1q).astype(BF16),
        "Wp2T": np.ascontiguousarray(Wp2Tb).astype(BF16),
        "Wa1T": np.ascontiguousarray(
            Wa1f.T.reshape(KA1, 128, d["AHID"]).transpose(1, 0, 2)).astype(BF16),
        "Wa2T": np.ascontiguousarray(
            Wa2.T.reshape(AM, 128, d["DIM"]).transpose(1, 0, 2)).astype(BF16),
        "WeT": np.ascontiguousarray(
            We.T.reshape(DM, 128, d["DIM"]).transpose(1, 0, 2)).astype(BF16),
        "ba1f": colsplit(ba1f, AM),
        "bef": colsplit((We @ bv + be).astype(f32), DM),
        "iota32": np.ascontiguousarray(
            np.broadcast_to(np.arange(d["N"], dtype=np.uint32)[None, :],
                            (QT, d["N"]))),
    }

    # hi/lo bf16 split of pos, |p_j|^2 and |p_i|^2 for the centered distance
    # matmul: dneg ~= -d  (top values near 0 -> fp16 eviction is precise)
    sq = (pos * pos).sum(axis=1).astype(f32)              # (B, N)
    pos_hi = pos.astype(BF16)
    pos_lo = (pos - pos_hi.astype(f32)).astype(BF16)
    sq_hi = sq.astype(BF16)
    sq_lo = (sq - sq_hi.astype(f32)).astype(BF16)
    paug_rhs = np.concatenate(
        [2.0 * pos_hi.astype(f32), 2.0 * pos_lo.astype(f32),
         2.0 * pos_hi.astype(f32), -sq_hi.astype(f32)[:, None, :],
         -sq_lo.astype(f32)[:, None, :],
         -np.ones((d["B"], 2, d["N"]), f32)], 1).astype(BF16)
    # (pos, 1) replicated into every 16-partition block so each gpsimd
    # core's gather block reads its own copy of the table
    pos16 = np.zeros((d["B"], QT, d["N"]), f32)
    for j in range(QT // 16):
        pos16[:, 16 * j:16 * j + 3] = pos
        pos16[:, 16 * j + 3] = 1.0

    in_maps = []
    for cid in range(ncores):
        n0 = cid * d["NLOC"]
        n1 = n0 + d["NLOC"]
        m = dict(common)
        m["key_r"] = np.ascontiguousarray(key[:, :, n0:n1, :]).reshape(
            d["B"], d["CIN"], d["NLOC"] * d["KK"]).astype(BF16)
        m["val_r"] = np.ascontiguousarray(values[:, :, n0:n1, :]).reshape(
            d["B"], d["CIN"], d["NLOC"] * d["KK"]).astype(BF16)
        m["paug_lhs"] = np.ascontiguousarray(np.concatenate(
            [pos_hi.astype(f32)[:, :, n0:n1], pos_hi.astype(f32)[:, :, n0:n1],
             pos_lo.astype(f32)[:, :, n0:n1],
             np.ones((d["B"], 2, d["NLOC"]), f32),
             sq_hi.astype(f32)[:, None, n0:n1],
             sq_lo.astype(f32)[:, None, n0:n1]], 1)).astype(BF16)
        m["paug_rhs"] = paug_rhs
        m["pos16"] = pos16
        in_maps.append(m)
    return in_maps


_NC_CACHE = {}


def _get_nc(dims_key):
    if dims_key not in _NC_CACHE:
        _NC_CACHE[dims_key] = build_nc(_dims_full())
    return _NC_CACHE[dims_key]


def kernel(**inputs):
    from concourse.bass_utils import run_bass_kernel_spmd
    dims = _dims_full()
    nc = _get_nc("full")
    in_maps = host_prepare(inputs, dims)
    res = run_bass_kernel_spmd(nc, in_maps, core_ids=list(range(NCORES)))
    outs = [r["out"].astype(np.float32) for r in res.results]
    return np.concatenate(outs, axis=2)
